# revision 3
# baseline (speedup 1.0000x reference)
"""ChebyConv (K=6) GNN kernel for 8 Trainium2 NeuronCores.

Strategy (data-parallel over batch, one batch element per core):
  - Host: sort nodes by in-degree (desc), relabel; build padded ELL edge
    structure; emit per-round gather index/weight arrays. Rounds: round j
    handles the j-th incoming edge of every node that has one; because
    nodes are degree-sorted, round j's destinations are a contiguous
    prefix of node positions -> the scatter side of spmm becomes wide
    contiguous vector adds into an SBUF-resident accumulator.
  - Pair-table trick: T tables are stored as [1 + npos/2, 128] f32 where
    row r holds positions (r-1) and (r-1 + npos/2). One 512-byte gather
    descriptor per edge (int16 idx covers all positions), and a per-slot
    interleaved weight vector (zero on the unused half) folds the
    half-select into the DVE multiply. Halves DMA cost vs the two-window
    256B double gather.
  - Device (per core): agg [128, NB*64] f32 in SBUF. Per hop: stream
    gather chunks, msg = sA * wint (both halves), fold halves, accumulate
    on DVE. Then T_next = 2*agg - T_prev (streamed), written to HBM.
    Final pass: out = sum_k T_k @ W[k] via PE transpose + PSUM matmuls;
    out dumped partition-major, unscrambled on host.
  - Host: unpermute rows, add bias.
"""

import math
import os
from dataclasses import dataclass, field

import numpy as np

import concourse.bacc as bacc
import concourse.bass as bass
import concourse.mybir as mybir
import concourse.tile as tile
from concourse.masks import make_identity

dt = mybir.dt

# ---------------------------------------------------------------- config

N_NODES = 50000
N_EDGES = 800000
BATCH = 8
CH = 64
K = 6
N_CORES = 8


@dataclass
class Cfg:
    n: int = N_NODES          # real nodes
    c: int = CH               # channels
    k: int = K                # chebyshev order
    chunk_bands: int = 32     # bands per gather chunk
    grp: int = 4              # bands per T_next/final group

    @property
    def npos(self) -> int:    # padded positions (multiple of 256 so the
        return ((self.n + 255) // 256) * 256  # pair-table half is band-aligned

    @property
    def nb(self) -> int:      # bands
        return self.npos // 128

    @property
    def half(self) -> int:    # positions per table half
        return self.npos // 2

    @property
    def hband(self) -> int:   # bands per half
        return self.nb // 2

    @property
    def nrows(self) -> int:   # pair-table rows: row 0 zero | 1..half data
        return self.half + 1


@dataclass
class Prep:
    cfg: Cfg
    order: np.ndarray         # position -> original node id
    idx: np.ndarray           # [128, TL] int16 (pair-table row per slot)
    wv: np.ndarray            # [128, 2*TB] f32 (interleaved half weights)
    # per chunk: (bands, agg_band_off, icol, wcol, first_touch)
    chunks: list = field(default_factory=list)
    tl: int = 0
    tb: int = 0


def prepare(cfg: Cfg, edge_index: np.ndarray, edge_weight: np.ndarray) -> Prep:
    n, npos, H = cfg.n, cfg.npos, cfg.half
    dst = np.asarray(edge_index[0], dtype=np.int64)
    src = np.asarray(edge_index[1], dtype=np.int64)
    w = np.asarray(edge_weight, dtype=np.float32)
    e = dst.shape[0]

    deg = np.bincount(dst, minlength=n)
    order = np.argsort(-deg, kind="stable")
    posof = np.empty(n, dtype=np.int64)
    posof[order] = np.arange(n)

    dstpos = posof[dst]
    eo = np.argsort(dstpos, kind="stable")
    sd = dstpos[eo]
    spos = posof[src[eo]]                           # src position (0-based)
    sw = w[eo]
    counts = np.bincount(sd, minlength=npos)
    starts = np.zeros(npos, dtype=np.int64)
    np.cumsum(counts[:-1], out=starts[1:])
    j = np.arange(e) - starts[sd]

    maxdeg = int(deg.max()) if e else 1
    # ELL: row index into pair table (0 = zero row), half bit, weight
    ell_row = np.zeros((npos, maxdeg), dtype=np.int64)
    ell_half = np.zeros((npos, maxdeg), dtype=np.int64)
    ell_w = np.zeros((npos, maxdeg), dtype=np.float32)
    ell_row[sd, j] = 1 + np.where(spos < H, spos, spos - H)
    ell_half[sd, j] = (spos >= H).astype(np.int64)
    ell_w[sd, j] = sw

    degpos = np.zeros(npos, dtype=np.int64)
    degpos[:n] = deg[order]

    idx_cols, w_cols, chunks = [], [], []
    icol = wcol = 0
    for jj in range(maxdeg):
        cj = int((degpos > jj).sum()) if jj > 0 else npos
        bj = (cj + 127) // 128
        b0 = 0
        while b0 < bj:
            bands = min(cfg.chunk_bands, bj - b0)
            s0, s1 = b0 * 128, (b0 + bands) * 128
            rows = ell_row[s0:s1, jj]                 # [S]
            halves = ell_half[s0:s1, jj]
            ws = ell_w[s0:s1, jj]
            S = s1 - s0
            L = S // 16
            ia = rows.astype(np.int16)
            idx_cols.append(np.tile(ia.reshape(L, 16).T, (8, 1)))
            # interleaved weights: wint[p, 2b+h] = w(slot b*128+p) if half==h
            wz = np.zeros((S, 2), dtype=np.float32)
            wz[np.arange(S), halves] = ws
            w_cols.append(wz.reshape(bands, 128, 2).transpose(1, 0, 2).reshape(128, bands * 2))
            chunks.append((bands, b0, icol, wcol, jj == 0))
            icol += L
            wcol += 2 * bands
            b0 += bands

    p = Prep(
        cfg=cfg,
        order=order,
        idx=np.ascontiguousarray(np.concatenate(idx_cols, axis=1)),
        wv=np.ascontiguousarray(np.concatenate(w_cols, axis=1).astype(np.float32)),
        chunks=chunks,
        tl=icol,
        tb=wcol,
    )
    return p


def make_x_dev(cfg: Cfg, prep: Prep, xb: np.ndarray) -> np.ndarray:
    """[n, c] original order -> [nrows, 2c] pair-table (sorted, zero-padded)."""
    H = cfg.half
    xs = np.zeros((cfg.npos, cfg.c), dtype=np.float32)
    xs[: cfg.n] = xb[prep.order]
    out = np.zeros((cfg.nrows, 2 * cfg.c), dtype=np.float32)
    out[1:, : cfg.c] = xs[:H]
    out[1:, cfg.c :] = xs[H:]
    return out


# ---------------------------------------------------------- numpy emulator


def emulate(cfg: Cfg, prep: Prep, xb: np.ndarray, W: np.ndarray) -> np.ndarray:
    """Bit-faithful numpy model of the device program (per batch element)."""
    c, H = cfg.c, cfg.half
    tabs = [make_x_dev(cfg, prep, xb)]
    agg = np.zeros((cfg.npos, c), dtype=np.float32)
    for k in range(1, cfg.k):
        srct = tabs[k - 1]
        for bands, b0, icol, wcol, first in prep.chunks:
            S = bands * 128
            L = S // 16
            ia = prep.idx[:16, icol : icol + L].T.reshape(S)
            wi = (
                prep.wv[:, wcol : wcol + 2 * bands]
                .reshape(128, bands, 2)
                .transpose(1, 0, 2)
                .reshape(S, 2)
            )
            g = srct[ia]                              # [S, 2c]
            msg = g[:, :c] * wi[:, 0:1] + g[:, c:] * wi[:, 1:2]
            sl = slice(b0 * 128, b0 * 128 + S)
            if first:
                agg[sl] = msg
            else:
                agg[sl] += msg
        if k == 1:
            tnext = agg.copy()
        else:
            prev = np.concatenate([tabs[k - 2][1:, :c], tabs[k - 2][1:, c:]], axis=0)
            tnext = 2.0 * agg - prev
        tab = np.zeros((cfg.nrows, 2 * c), dtype=np.float32)
        tab[1:, :c] = tnext[:H]
        tab[1:, c:] = tnext[H:]
        tabs.append(tab)
    out = np.zeros((cfg.npos, c), dtype=np.float32)
    for k in range(cfg.k):
        tk = np.concatenate([tabs[k][1:, :c], tabs[k][1:, c:]], axis=0)
        out += tk @ W[k]
    return out


# ----------------------------------------------------------- bass program


def build_program(cfg: Cfg, prep: Prep, variant: str = "full"):
    # variant: "full" | "nogather" | "nodve" | "nofinal" | "hops1"
    do_gather = variant != "nogather"
    do_dve = variant != "nodve"
    do_final = variant not in ("nofinal", "hops1") and not variant.startswith("x")
    n_hops = 1 if variant == "hops1" else None
    repeats = int(variant[1:]) if variant.startswith("x") else 1
    nc = bacc.Bacc(None, target_bir_lowering=False, debug=False)
    c = cfg.c

    x = nc.dram_tensor("x", [cfg.nrows, 2 * c], dt.float32, kind="ExternalInput")
    ia_d = nc.dram_tensor("idx", [128, prep.tl], dt.int16, kind="ExternalInput")
    wv_d = nc.dram_tensor("wv", [128, prep.tb], dt.float32, kind="ExternalInput")
    W_d = nc.dram_tensor("Wmat", [c, cfg.k * c], dt.float32, kind="ExternalInput")
    out_d = nc.dram_tensor("out", [128, cfg.nb * c], dt.float32, kind="ExternalOutput")

    tabs = [x] + [
        nc.dram_tensor(f"T{k}", [cfg.nrows, 2 * c], dt.float32) for k in range(1, cfg.k)
    ]

    cb, grp, nb, hband = cfg.chunk_bands, cfg.grp, cfg.nb, cfg.hband
    n_grp = (nb + grp - 1) // grp

    def rowgrp(tab, band0, nbands):
        """DRAM view [p, g, c] of positions band0*128 .. (band0+nbands)*128.

        Band range must not straddle the half boundary (grp divides hband).
        """
        if band0 < hband:
            r0 = 1 + band0 * 128
            return tab[r0 : r0 + nbands * 128, 0:c].rearrange(
                "(g p) c -> p g c", p=128
            )
        r0 = 1 + (band0 - hband) * 128
        return tab[r0 : r0 + nbands * 128, c : 2 * c].rearrange(
            "(g p) c -> p g c", p=128
        )

    with tile.TileContext(nc) as tc:
        with (
            tc.tile_pool(name="const", bufs=1) as cst,
            tc.tile_pool(name="aggp", bufs=1) as aggp,
            tc.tile_pool(name="idxp", bufs=3) as idxp,
            tc.tile_pool(name="wp", bufs=3) as wp,
            tc.tile_pool(name="stg", bufs=2) as stg,
            tc.tile_pool(name="prv", bufs=2) as prvp,
            tc.tile_pool(name="ld", bufs=2) as ldp,
            tc.tile_pool(name="outp", bufs=2) as outp,
            tc.tile_pool(name="tTp", bufs=2) as tTp,
            tc.tile_pool(name="ps", bufs=2, space="PSUM") as psp,
            tc.tile_pool(name="pso", bufs=2, space="PSUM") as psop,
        ):
            ident = cst.tile([128, 128], dt.float32)
            make_identity(nc, ident[:])
            W_t = cst.tile([c, cfg.k * c], dt.float32)
            nc.sync.dma_start(W_t[:], W_d[:])
            zt = cst.tile([128, 2 * c], dt.float32)
            nc.vector.memset(zt[:], 0.0)

            # zero row 0 of the T tables (gather target for padded slots)
            for t in tabs[1:]:
                nc.sync.dma_start(t[0:1, :], zt[0:1, :])

            agg = aggp.tile([128, nb * c], dt.float32)
            if not do_dve:
                nc.gpsimd.memset(agg[:], 0.0)

            for rep in range(repeats):
              for k in range(1, (1 + n_hops) if n_hops else cfg.k):
                  srct, dstt = tabs[k - 1], tabs[k]
                  for bands, b0, icol, wcol, first in prep.chunks:
                      S = bands * 128
                      L = S // 16
                      ia_t = idxp.tile([128, cb * 8], dt.int16, tag="ia")
                      w_t = wp.tile([128, cb * 2], dt.float32, tag="w")
                      nc.sync.dma_start(ia_t[:, :L], ia_d[:, icol : icol + L])
                      nc.sync.dma_start(w_t[:, : 2 * bands], wv_d[:, wcol : wcol + 2 * bands])
                      sA = stg.tile([128, cb * 2 * c], dt.float32, tag="sA")
                      sA3 = sA[:, : bands * 2 * c].rearrange("p (b c) -> p b c", c=2 * c)
                      if do_gather:
                          nc.gpsimd.dma_gather(
                              out_ap=sA3,
                              in_ap=srct[0 : cfg.nrows, :],
                              idxs_ap=ia_t[:, :L],
                              num_idxs=S,
                              num_idxs_reg=S,
                              elem_size=2 * c,
                              single_packet=(S <= 1008),
                          )
                      a_sl = agg[:, b0 * c : (b0 + bands) * c]
                      a3 = a_sl.rearrange("p (b c) -> p b c", c=c)
                      sH = sA[:, : bands * 2 * c].rearrange("p (b c) -> p b c", c=c)
                      lo = sA3[:, :, 0:c]
                      hi = sA3[:, :, c : 2 * c]
                      w_bc = (
                          w_t[:, : 2 * bands].unsqueeze(2).to_broadcast([128, 2 * bands, c])
                      )
                      if not do_dve:
                          pass
                      elif first:
                          nc.vector.tensor_tensor(out=sH, in0=sH, in1=w_bc, op=mybir.AluOpType.mult)
                          nc.vector.tensor_add(a3, lo, hi)
                      else:
                          nc.vector.tensor_tensor(out=sH, in0=sH, in1=w_bc, op=mybir.AluOpType.mult)
                          nc.vector.tensor_add(lo, lo, hi)
                          nc.vector.tensor_add(a3, a3, lo)

                  # T_next = (k==1) ? agg : 2*agg - T_prev ; stream out
                  for g in range(n_grp):
                      gb = min(grp, nb - g * grp)
                      a_sl = agg[:, g * grp * c : (g * grp + gb) * c]
                      if k >= 2 and do_dve:
                          prvt = tabs[k - 2]
                          pv = prvp.tile([128, grp * c], dt.float32, tag="pv")
                          nc.sync.dma_start(
                              pv[:, : gb * c].rearrange("p (g c) -> p g c", c=c),
                              rowgrp(prvt, g * grp, gb),
                          )
                          nc.vector.scalar_tensor_tensor(
                              out=a_sl, in0=a_sl, scalar=2.0, in1=pv[:, : gb * c],
                              op0=mybir.AluOpType.mult, op1=mybir.AluOpType.subtract,
                          )
                      nc.sync.dma_start(
                          rowgrp(dstt, g * grp, gb),
                          a_sl.rearrange("p (g c) -> p g c", c=c),
                      )

            # final: out = sum_k T_k @ W_k
            for g in (range(n_grp) if do_final else []):
                gb = min(grp, nb - g * grp)
                tls = []
                for k in range(cfg.k):
                    tl = ldp.tile([128, grp, c], dt.float32, tag=f"ld{k}")
                    nc.sync.dma_start(tl[:, :gb, :], rowgrp(tabs[k], g * grp, gb))
                    tls.append(tl)
                o_t = outp.tile([128, grp * c], dt.float32, tag="ot")
                for i in range(gb):
                    psT = psp.tile([c, cfg.k * 128], dt.float32)
                    for k in range(cfg.k):
                        nc.tensor.transpose(
                            psT[:, k * 128 : (k + 1) * 128], tls[k][:, i, :], ident[:]
                        )
                    tT = tTp.tile([c, cfg.k * 128], dt.float32, tag="tT")
                    nc.vector.tensor_copy(tT[:], psT[:])
                    oP = psop.tile([128, c], dt.float32)
                    for k in range(cfg.k):
                        nc.tensor.matmul(
                            oP[:],
                            lhsT=tT[:, k * 128 : (k + 1) * 128],
                            rhs=W_t[:, k * c : (k + 1) * c],
                            start=(k == 0),
                            stop=(k == cfg.k - 1),
                        )
                    nc.scalar.copy(o_t[:, i * c : (i + 1) * c], oP[:])
                nc.sync.dma_start(
                    out_d[:, g * grp * c : (g * grp + gb) * c],
                    o_t[:, : gb * c],
                )

    nc.compile()
    return nc


# ------------------------------------------------------------------ entry

_CACHE = {}
LAST_RESULTS = None


def _install_ntff_hook():
    """The agent image's antenv package lacks axon_hooks; inject it and
    register the ctypes NTFF profile hook so trace=True works under axon."""
    import sys
    import types
    import ctypes
    import contextlib

    if "antenv.axon_hooks" in sys.modules:
        return
    mod = types.ModuleType("antenv.axon_hooks")
    state = {"hook": None}
    mod.set_axon_ntff_profile_hook = lambda h: state.__setitem__("hook", h)
    mod.get_axon_ntff_profile_hook = lambda: state["hook"]
    sys.modules["antenv.axon_hooks"] = mod

    so_path = "/opt/axon/libaxon_pjrt.so"
    try:
        lib = ctypes.CDLL(so_path)
        if not hasattr(lib, "axon_start_nrt_profile"):
            return
        lib.axon_start_nrt_profile.argtypes = [
            ctypes.POINTER(ctypes.c_int64),
            ctypes.c_size_t,
        ]
        lib.axon_start_nrt_profile.restype = ctypes.c_int64
        lib.axon_stop_nrt_profile.argtypes = [ctypes.c_char_p]
        lib.axon_stop_nrt_profile.restype = ctypes.c_int64

        @contextlib.contextmanager
        def _hook(output_dir, device_ids):
            import jax

            jax.devices()
            if device_ids:
                ids = (ctypes.c_int64 * len(device_ids))(*device_ids)
                rc = lib.axon_start_nrt_profile(ids, len(device_ids))
            else:
                rc = lib.axon_start_nrt_profile(None, 0)
            if rc != 0:
                raise RuntimeError(f"axon_start_nrt_profile rc={rc}")
            try:
                yield
            finally:
                n = lib.axon_stop_nrt_profile(str(output_dir).encode())
                print(f"ntff profile: {n} file(s) -> {output_dir}")

        mod.set_axon_ntff_profile_hook(_hook)
    except OSError:
        pass


def kernel(x, edge_index, edge_weight, W, bias):
    global LAST_RESULTS
    from concourse.bass_utils import run_bass_kernel_spmd

    x = np.asarray(x, dtype=np.float32)
    edge_index = np.asarray(edge_index)
    edge_weight = np.asarray(edge_weight, dtype=np.float32)
    W = np.asarray(W, dtype=np.float32)
    bias = np.asarray(bias, dtype=np.float32)

    b, n, c = x.shape
    cfg = Cfg(n=n, c=c, k=W.shape[0])

    key = ("v2", n, c, W.shape[0])
    if key in _CACHE:
        prep, nc = _CACHE[key]
        # prep depends on edges; rebuild if edges changed
        if not np.array_equal(prep._edge_sig, _edge_sig(edge_index, edge_weight)):
            prep = prepare(cfg, edge_index, edge_weight)
            prep._edge_sig = _edge_sig(edge_index, edge_weight)
            nc = build_program(cfg, prep)
            _CACHE[key] = (prep, nc)
    else:
        prep = prepare(cfg, edge_index, edge_weight)
        prep._edge_sig = _edge_sig(edge_index, edge_weight)
        nc = build_program(cfg, prep)
        _CACHE[key] = (prep, nc)

    Wflat = np.ascontiguousarray(
        W.transpose(1, 0, 2).reshape(c, cfg.k * c).astype(np.float32)
    )
    in_maps = []
    for core in range(N_CORES):
        bb = core % b
        in_maps.append(
            {
                "x": make_x_dev(cfg, prep, x[bb]),
                "idx": prep.idx,
                "wv": prep.wv,
                "Wmat": Wflat,
            }
        )

    trace = bool(os.environ.get("KBENCH_TRACE"))
    if trace:
        _install_ntff_hook()
    tmpdir = os.environ.get("KBENCH_TMPDIR") or None
    try:
        res = run_bass_kernel_spmd(
            nc, in_maps, core_ids=list(range(N_CORES)), trace=trace, tmpdir=tmpdir,
        )
    except Exception:
        if not trace:
            raise
        res = run_bass_kernel_spmd(
            nc, in_maps, core_ids=list(range(N_CORES)), trace=False,
        )
    LAST_RESULTS = res

    out = np.zeros((b, n, c), dtype=np.float32)
    for bb in range(b):
        o = res.results[bb]["out"]                    # [128, nb*c] partition-major
        o = o.reshape(128, cfg.nb, c).transpose(1, 0, 2).reshape(cfg.npos, c)
        out[bb][prep.order] = o[:n]
    out += bias[None, None, :]
    return out


def _edge_sig(edge_index, edge_weight):
    return np.concatenate(
        [np.asarray(edge_index, dtype=np.int64).ravel()[:64].astype(np.float64),
         np.asarray(edge_weight, dtype=np.float64).ravel()[:64]]
    )


# revision 10
# speedup vs baseline: 1.6185x; 1.6185x over previous
"""ChebyConv (K=6) GNN kernel for 8 Trainium2 NeuronCores.

Strategy (data-parallel over batch, one batch element per core):
  - Host: sort nodes by in-degree (desc), relabel; build padded ELL edge
    structure; emit per-round gather index/weight arrays. Rounds: round j
    handles the j-th incoming edge of every node that has one; because
    nodes are degree-sorted, round j's destinations are a contiguous
    prefix of node positions -> the scatter side of spmm becomes wide
    contiguous vector adds into an SBUF-resident accumulator.
  - Pair-table trick: T tables are stored as [1 + npos/2, 128] f32 where
    row r holds positions (r-1) and (r-1 + npos/2). One 512-byte gather
    descriptor per edge (int16 idx covers all positions), and a per-slot
    interleaved weight vector (zero on the unused half) folds the
    half-select into the DVE multiply. Halves DMA cost vs the two-window
    256B double gather.
  - Device (per core): agg [128, NB*64] f32 in SBUF. Per hop: stream
    gather chunks, msg = sA * wint (both halves), fold halves, accumulate
    on DVE. Then T_next = 2*agg - T_prev (streamed), written to HBM.
    Final pass: out = sum_k T_k @ W[k] via PE transpose + PSUM matmuls;
    out dumped partition-major, unscrambled on host.
  - Host: unpermute rows, add bias.
"""

import math
import os
from dataclasses import dataclass, field

import numpy as np

import concourse.bacc as bacc
import concourse.bass as bass
import concourse.mybir as mybir
import concourse.tile as tile
from concourse.masks import make_identity

dt = mybir.dt

# ---------------------------------------------------------------- config

N_NODES = 50000
N_EDGES = 800000
BATCH = 8
CH = 64
K = 6
N_CORES = 8


@dataclass
class Cfg:
    n: int = N_NODES          # real nodes
    c: int = CH               # channels
    k: int = K                # chebyshev order
    chunk_bands: int = 32     # bands per gather chunk
    grp: int = 4              # bands per T_next/final group

    @property
    def npos(self) -> int:    # padded positions (multiple of 256 so the
        return ((self.n + 255) // 256) * 256  # pair-table half is band-aligned

    @property
    def nb(self) -> int:      # bands
        return self.npos // 128

    @property
    def half(self) -> int:    # positions per table half
        return self.npos // 2

    @property
    def hband(self) -> int:   # bands per half
        return self.nb // 2

    @property
    def nrows(self) -> int:   # pair-table rows: row 0 zero | 1..half data
        return self.half + 1


@dataclass
class Prep:
    cfg: Cfg
    order: np.ndarray         # position -> original node id
    idx: np.ndarray           # [128, TL] int16 (pair-table row per slot)
    wv: np.ndarray            # [128, 2*TB] f32 (interleaved half weights)
    # per chunk: (bands, agg_band_off, icol, wcol, first_touch)
    chunks: list = field(default_factory=list)
    tl: int = 0
    tb: int = 0


def prepare(cfg: Cfg, edge_index: np.ndarray, edge_weight: np.ndarray) -> Prep:
    n, npos, H = cfg.n, cfg.npos, cfg.half
    dst = np.asarray(edge_index[0], dtype=np.int64)
    src = np.asarray(edge_index[1], dtype=np.int64)
    w = np.asarray(edge_weight, dtype=np.float32)
    e = dst.shape[0]

    deg = np.bincount(dst, minlength=n)
    order = np.argsort(-deg, kind="stable")
    posof = np.empty(n, dtype=np.int64)
    posof[order] = np.arange(n)

    dstpos = posof[dst]
    eo = np.argsort(dstpos, kind="stable")
    sd = dstpos[eo]
    spos = posof[src[eo]]                           # src position (0-based)
    sw = w[eo]
    counts = np.bincount(sd, minlength=npos)
    starts = np.zeros(npos, dtype=np.int64)
    np.cumsum(counts[:-1], out=starts[1:])
    j = np.arange(e) - starts[sd]

    maxdeg = int(deg.max()) if e else 1
    # ELL: row index into pair table (0 = zero row), half bit, weight
    ell_row = np.zeros((npos, maxdeg), dtype=np.int64)
    ell_half = np.zeros((npos, maxdeg), dtype=np.int64)
    ell_w = np.zeros((npos, maxdeg), dtype=np.float32)
    ell_row[sd, j] = 1 + np.where(spos < H, spos, spos - H)
    ell_half[sd, j] = (spos >= H).astype(np.int64)
    ell_w[sd, j] = sw

    degpos = np.zeros(npos, dtype=np.int64)
    degpos[:n] = deg[order]

    idx_cols, w_cols, chunks = [], [], []
    icol = wcol = 0
    for jj in range(maxdeg):
        cj = int((degpos > jj).sum()) if jj > 0 else npos
        bj = (cj + 127) // 128
        b0 = 0
        while b0 < bj:
            bands = min(cfg.chunk_bands, bj - b0)
            s0, s1 = b0 * 128, (b0 + bands) * 128
            rows = ell_row[s0:s1, jj]                 # [S]
            halves = ell_half[s0:s1, jj]
            ws = ell_w[s0:s1, jj]
            S = s1 - s0
            L = S // 16
            ia = rows.astype(np.int16)
            idx_cols.append(np.tile(ia.reshape(L, 16).T, (8, 1)))
            # interleaved weights: wint[p, 2b+h] = w(slot b*128+p) if half==h
            wz = np.zeros((S, 2), dtype=np.float32)
            wz[np.arange(S), halves] = ws
            w_cols.append(wz.reshape(bands, 128, 2).transpose(1, 0, 2).reshape(128, bands * 2))
            chunks.append((bands, b0, icol, wcol, jj == 0))
            icol += L
            wcol += 2 * bands
            b0 += bands

    p = Prep(
        cfg=cfg,
        order=order,
        idx=np.ascontiguousarray(np.concatenate(idx_cols, axis=1)),
        wv=np.ascontiguousarray(np.concatenate(w_cols, axis=1).astype(np.float32)),
        chunks=chunks,
        tl=icol,
        tb=wcol,
    )
    return p


def make_x_dev(cfg: Cfg, prep: Prep, xb: np.ndarray) -> np.ndarray:
    """[n, c] original order -> [nrows, 2c] pair-table (sorted, zero-padded)."""
    H = cfg.half
    xs = np.zeros((cfg.npos, cfg.c), dtype=np.float32)
    xs[: cfg.n] = xb[prep.order]
    out = np.zeros((cfg.nrows, 2 * cfg.c), dtype=np.float32)
    out[1:, : cfg.c] = xs[:H]
    out[1:, cfg.c :] = xs[H:]
    return out


# ---------------------------------------------------------- numpy emulator


def emulate(cfg: Cfg, prep: Prep, xb: np.ndarray, W: np.ndarray) -> np.ndarray:
    """Bit-faithful numpy model of the device program (per batch element)."""
    c, H = cfg.c, cfg.half
    tabs = [make_x_dev(cfg, prep, xb)]
    agg = np.zeros((cfg.npos, c), dtype=np.float32)
    for k in range(1, cfg.k):
        srct = tabs[k - 1]
        for bands, b0, icol, wcol, first in prep.chunks:
            S = bands * 128
            L = S // 16
            ia = prep.idx[:16, icol : icol + L].T.reshape(S)
            wi = (
                prep.wv[:, wcol : wcol + 2 * bands]
                .reshape(128, bands, 2)
                .transpose(1, 0, 2)
                .reshape(S, 2)
            )
            g = srct[ia]                              # [S, 2c]
            msg = g[:, :c] * wi[:, 0:1] + g[:, c:] * wi[:, 1:2]
            sl = slice(b0 * 128, b0 * 128 + S)
            if first:
                agg[sl] = msg
            else:
                agg[sl] += msg
        if k == 1:
            tnext = agg.copy()
        else:
            prev = np.concatenate([tabs[k - 2][1:, :c], tabs[k - 2][1:, c:]], axis=0)
            tnext = 2.0 * agg - prev
        tab = np.zeros((cfg.nrows, 2 * c), dtype=np.float32)
        tab[1:, :c] = tnext[:H]
        tab[1:, c:] = tnext[H:]
        tabs.append(tab)
    out = np.zeros((cfg.npos, c), dtype=np.float32)
    for k in range(cfg.k):
        tk = np.concatenate([tabs[k][1:, :c], tabs[k][1:, c:]], axis=0)
        out += tk @ W[k]
    return out


# ----------------------------------------------------------- bass program


def build_program(cfg: Cfg, prep: Prep, variant: str = "full"):
    # variant: "full" | "nogather" | "nodve" | "nofinal" | "hops1"
    do_gather = variant != "nogather"
    do_dve = variant != "nodve"
    do_final = variant not in ("nofinal", "hops1") and not variant.startswith("x")
    n_hops = 1 if variant == "hops1" else None
    repeats = int(variant[1:]) if variant.startswith("x") else 1
    nc = bacc.Bacc(
        None, target_bir_lowering=False, debug=False, num_swdge_queues=4
    )
    c = cfg.c

    x = nc.dram_tensor("x", [cfg.nrows, 2 * c], dt.float32, kind="ExternalInput")
    ia_d = nc.dram_tensor("idx", [128, prep.tl], dt.int16, kind="ExternalInput")
    wv_d = nc.dram_tensor("wv", [128, prep.tb], dt.float32, kind="ExternalInput")
    W_d = nc.dram_tensor("Wmat", [c, cfg.k * c], dt.float32, kind="ExternalInput")
    out_d = nc.dram_tensor("out", [128, cfg.nb * c], dt.float32, kind="ExternalOutput")

    tabs = [x] + [
        nc.dram_tensor(f"T{k}", [cfg.nrows, 2 * c], dt.float32) for k in range(1, cfg.k)
    ]

    cb, grp, nb, hband = cfg.chunk_bands, cfg.grp, cfg.nb, cfg.hband
    n_grp = (nb + grp - 1) // grp

    def rowgrp(tab, band0, nbands):
        """DRAM view [p, g, c] of positions band0*128 .. (band0+nbands)*128.

        Band range must not straddle the half boundary (grp divides hband).
        """
        if band0 < hband:
            r0 = 1 + band0 * 128
            return tab[r0 : r0 + nbands * 128, 0:c].rearrange(
                "(g p) c -> p g c", p=128
            )
        r0 = 1 + (band0 - hband) * 128
        return tab[r0 : r0 + nbands * 128, c : 2 * c].rearrange(
            "(g p) c -> p g c", p=128
        )

    with tile.TileContext(nc) as tc:
        with (
            tc.tile_pool(name="const", bufs=1) as cst,
            tc.tile_pool(name="aggp", bufs=1) as aggp,
            tc.tile_pool(name="idxp", bufs=6) as idxp,
            tc.tile_pool(name="wp", bufs=6) as wp,
            tc.tile_pool(name="stg", bufs=4) as stg,
            tc.tile_pool(name="prv", bufs=2) as prvp,
            tc.tile_pool(name="ld", bufs=2) as ldp,
            tc.tile_pool(name="outp", bufs=2) as outp,
            tc.tile_pool(name="tTp", bufs=2) as tTp,
            tc.tile_pool(name="ps", bufs=2, space="PSUM") as psp,
            tc.tile_pool(name="pso", bufs=2, space="PSUM") as psop,
        ):
            ident = cst.tile([128, 128], dt.float32)
            make_identity(nc, ident[:])
            W_t = cst.tile([c, cfg.k * c], dt.float32)
            nc.sync.dma_start(W_t[:], W_d[:])
            zt = cst.tile([128, 2 * c], dt.float32)
            nc.vector.memset(zt[:], 0.0)

            # zero row 0 of the T tables (gather target for padded slots)
            for t in tabs[1:]:
                nc.sync.dma_start(t[0:1, :], zt[0:1, :])

            agg = aggp.tile([128, nb * c], dt.float32)
            if not do_dve:
                nc.gpsimd.memset(agg[:], 0.0)

            for rep in range(repeats):
              for k in range(1, (1 + n_hops) if n_hops else cfg.k):
                  srct, dstt = tabs[k - 1], tabs[k]
                  for ci, (bands, b0, icol, wcol, first) in enumerate(prep.chunks):
                      S = bands * 128
                      L = S // 16
                      ia_t = idxp.tile([128, cb * 8], dt.int16, tag="ia")
                      w_t = wp.tile([128, cb * 2], dt.float32, tag="w")
                      nc.sync.dma_start(ia_t[:, :L], ia_d[:, icol : icol + L])
                      nc.sync.dma_start(w_t[:, : 2 * bands], wv_d[:, wcol : wcol + 2 * bands])
                      sA = stg.tile([128, cb * 2 * c], dt.float32, tag="sA")
                      sA3 = sA[:, : bands * 2 * c].rearrange("p (b c) -> p b c", c=2 * c)
                      if do_gather:
                          nc.gpsimd.dma_gather(
                              out_ap=sA3,
                              in_ap=srct[0 : cfg.nrows, :],
                              idxs_ap=ia_t[:, :L],
                              num_idxs=S,
                              num_idxs_reg=S,
                              elem_size=2 * c,
                              single_packet=(S <= 1008),
                              queue_num=ci % 4,
                          )
                      a_sl = agg[:, b0 * c : (b0 + bands) * c]
                      a3 = a_sl.rearrange("p (b c) -> p b c", c=c)
                      sH = sA[:, : bands * 2 * c].rearrange("p (b c) -> p b c", c=c)
                      lo = sA3[:, :, 0:c]
                      hi = sA3[:, :, c : 2 * c]
                      w_bc = (
                          w_t[:, : 2 * bands].unsqueeze(2).to_broadcast([128, 2 * bands, c])
                      )
                      if not do_dve:
                          pass
                      elif first:
                          nc.vector.tensor_tensor(out=sH, in0=sH, in1=w_bc, op=mybir.AluOpType.mult)
                          nc.vector.tensor_add(a3, lo, hi)
                      else:
                          nc.vector.tensor_tensor(out=sH, in0=sH, in1=w_bc, op=mybir.AluOpType.mult)
                          nc.vector.tensor_add(lo, lo, hi)
                          nc.vector.tensor_add(a3, a3, lo)

                  # T_next = (k==1) ? agg : 2*agg - T_prev ; stream out
                  for g in range(n_grp):
                      gb = min(grp, nb - g * grp)
                      a_sl = agg[:, g * grp * c : (g * grp + gb) * c]
                      if k >= 2 and do_dve:
                          prvt = tabs[k - 2]
                          pv = prvp.tile([128, grp * c], dt.float32, tag="pv")
                          nc.sync.dma_start(
                              pv[:, : gb * c].rearrange("p (g c) -> p g c", c=c),
                              rowgrp(prvt, g * grp, gb),
                          )
                          nc.vector.scalar_tensor_tensor(
                              out=a_sl, in0=a_sl, scalar=2.0, in1=pv[:, : gb * c],
                              op0=mybir.AluOpType.mult, op1=mybir.AluOpType.subtract,
                          )
                      nc.sync.dma_start(
                          rowgrp(dstt, g * grp, gb),
                          a_sl.rearrange("p (g c) -> p g c", c=c),
                      )

            # final: out = sum_k T_k @ W_k
            for g in (range(n_grp) if do_final else []):
                gb = min(grp, nb - g * grp)
                tls = []
                for k in range(cfg.k):
                    tl = ldp.tile([128, grp, c], dt.float32, tag=f"ld{k}")
                    nc.sync.dma_start(tl[:, :gb, :], rowgrp(tabs[k], g * grp, gb))
                    tls.append(tl)
                o_t = outp.tile([128, grp * c], dt.float32, tag="ot")
                for i in range(gb):
                    psT = psp.tile([c, cfg.k * 128], dt.float32)
                    for k in range(cfg.k):
                        nc.tensor.transpose(
                            psT[:, k * 128 : (k + 1) * 128], tls[k][:, i, :], ident[:]
                        )
                    tT = tTp.tile([c, cfg.k * 128], dt.float32, tag="tT")
                    nc.vector.tensor_copy(tT[:], psT[:])
                    oP = psop.tile([128, c], dt.float32)
                    for k in range(cfg.k):
                        nc.tensor.matmul(
                            oP[:],
                            lhsT=tT[:, k * 128 : (k + 1) * 128],
                            rhs=W_t[:, k * c : (k + 1) * c],
                            start=(k == 0),
                            stop=(k == cfg.k - 1),
                        )
                    nc.scalar.copy(o_t[:, i * c : (i + 1) * c], oP[:])
                nc.sync.dma_start(
                    out_d[:, g * grp * c : (g * grp + gb) * c],
                    o_t[:, : gb * c],
                )

    nc.compile()
    return nc


# ------------------------------------------------------------------ entry

_CACHE = {}
LAST_RESULTS = None


def _install_ntff_hook():
    """The agent image's antenv package lacks axon_hooks; inject it and
    register the ctypes NTFF profile hook so trace=True works under axon."""
    import sys
    import types
    import ctypes
    import contextlib

    if "antenv.axon_hooks" in sys.modules:
        return
    mod = types.ModuleType("antenv.axon_hooks")
    state = {"hook": None}
    mod.set_axon_ntff_profile_hook = lambda h: state.__setitem__("hook", h)
    mod.get_axon_ntff_profile_hook = lambda: state["hook"]
    sys.modules["antenv.axon_hooks"] = mod

    so_path = "/opt/axon/libaxon_pjrt.so"
    try:
        lib = ctypes.CDLL(so_path)
        if not hasattr(lib, "axon_start_nrt_profile"):
            return
        lib.axon_start_nrt_profile.argtypes = [
            ctypes.POINTER(ctypes.c_int64),
            ctypes.c_size_t,
        ]
        lib.axon_start_nrt_profile.restype = ctypes.c_int64
        lib.axon_stop_nrt_profile.argtypes = [ctypes.c_char_p]
        lib.axon_stop_nrt_profile.restype = ctypes.c_int64

        @contextlib.contextmanager
        def _hook(output_dir, device_ids):
            import jax

            jax.devices()
            if device_ids:
                ids = (ctypes.c_int64 * len(device_ids))(*device_ids)
                rc = lib.axon_start_nrt_profile(ids, len(device_ids))
            else:
                rc = lib.axon_start_nrt_profile(None, 0)
            if rc != 0:
                raise RuntimeError(f"axon_start_nrt_profile rc={rc}")
            try:
                yield
            finally:
                n = lib.axon_stop_nrt_profile(str(output_dir).encode())
                print(f"ntff profile: {n} file(s) -> {output_dir}")

        mod.set_axon_ntff_profile_hook(_hook)
    except OSError:
        pass


def kernel(x, edge_index, edge_weight, W, bias):
    global LAST_RESULTS
    from concourse.bass_utils import run_bass_kernel_spmd

    x = np.asarray(x, dtype=np.float32)
    edge_index = np.asarray(edge_index)
    edge_weight = np.asarray(edge_weight, dtype=np.float32)
    W = np.asarray(W, dtype=np.float32)
    bias = np.asarray(bias, dtype=np.float32)

    b, n, c = x.shape
    cfg = Cfg(n=n, c=c, k=W.shape[0])

    key = ("v4", n, c, W.shape[0])
    if key in _CACHE:
        prep, nc = _CACHE[key]
        # prep depends on edges; rebuild if edges changed
        if not np.array_equal(prep._edge_sig, _edge_sig(edge_index, edge_weight)):
            prep = prepare(cfg, edge_index, edge_weight)
            prep._edge_sig = _edge_sig(edge_index, edge_weight)
            nc = build_program(cfg, prep)
            _CACHE[key] = (prep, nc)
    else:
        prep = prepare(cfg, edge_index, edge_weight)
        prep._edge_sig = _edge_sig(edge_index, edge_weight)
        nc = build_program(cfg, prep)
        _CACHE[key] = (prep, nc)

    Wflat = np.ascontiguousarray(
        W.transpose(1, 0, 2).reshape(c, cfg.k * c).astype(np.float32)
    )
    in_maps = []
    for core in range(N_CORES):
        bb = core % b
        in_maps.append(
            {
                "x": make_x_dev(cfg, prep, x[bb]),
                "idx": prep.idx,
                "wv": prep.wv,
                "Wmat": Wflat,
            }
        )

    trace = bool(os.environ.get("KBENCH_TRACE"))
    if trace:
        _install_ntff_hook()
    tmpdir = os.environ.get("KBENCH_TMPDIR") or None
    try:
        res = run_bass_kernel_spmd(
            nc, in_maps, core_ids=list(range(N_CORES)), trace=trace, tmpdir=tmpdir,
        )
    except Exception:
        if not trace:
            raise
        res = run_bass_kernel_spmd(
            nc, in_maps, core_ids=list(range(N_CORES)), trace=False,
        )
    LAST_RESULTS = res

    out = np.zeros((b, n, c), dtype=np.float32)
    for bb in range(b):
        o = res.results[bb]["out"]                    # [128, nb*c] partition-major
        o = o.reshape(128, cfg.nb, c).transpose(1, 0, 2).reshape(cfg.npos, c)
        out[bb][prep.order] = o[:n]
    out += bias[None, None, :]
    return out


def _edge_sig(edge_index, edge_weight):
    return np.concatenate(
        [np.asarray(edge_index, dtype=np.int64).ravel()[:64].astype(np.float64),
         np.asarray(edge_weight, dtype=np.float64).ravel()[:64]]
    )
